# revision 26
# baseline (speedup 1.0000x reference)
"""MixIT loss kernel for Trainium2 (raw Bass), 8-way data-parallel over batch.

Math: the loss depends only on Gram statistics of the stacked signals
D = [sources(8); mixtures(2)] over T=32000.  With b1_k = [a1_k; 0; -1] and
b0_k = [1-a1_k; -1; 0] (10-vectors), the per-combo noise energies are pure
quadratic forms in the 10x10 Gram G:
  ne1_k = b1_k^T G b1_k,   ne0_k = b0_k^T G b0_k,
  per_sample = 10/ln(10) * (ln(min_k ne0_k*ne1_k) - ln(E0*E1))
(the tau*E regularizers are ~2e-7 relative here and are dropped).

Key device trick: G never needs to be folded to 10x10.  The interleaved
layout R[p, b*100 + i*10 + s] = D[s, p*250 + b*10 + i] makes each 100-col
block's self-product a [100x100] PSUM Gram whose block-diagonal holds
i-resolved sub-Grams.  For TILED vectors u_j[(i,t)] = b_j[t],
  u_j^T (mask o M100) u_j = b_j^T G b_j
exactly, so with mc = mask o M100 (one DVE multiply) the combo stage is:
  P1 = mc^T U          (U [100, 512] constant: u1 | pad | u0 | e8 | e9)
  buf = P1 o U         (DVE elementwise, two column halves)
  P2 = ones100^T buf   -> [1, 512] = [ne1_k | ne0_k | E0 | E1]
  pk = ne1 o ne0;  res2 = [min_k pk, E0*E1]
step1/fold run as two 256-col matmuls each so the DVE Hadamard on half 0
overlaps the PE pass on half 1.  Host does ln/scale/mean in the gather that
averages the 8 cores.

Dataflow per core (one batch sample per core):
  - Host interleaves + casts to fp8e4m3 (quarter the HBM bytes of f32).
  - Input DMAs issue BEFORE the block: four growing data waves on the SP
    HWDGE queue (descriptor generation overlaps block entry), constants on
    ACT gated behind wave 0 so they never steal DMA-engine turns.
  - PE runs warm-up matmuls on a zeroed dummy tile while DMA is in flight
    (avoids the ~230ns pipeline cold-start), then 25 fp8 DoubleRow matmuls
    (each contracts TWO padded 64-col blocks at ~73ns) accumulate the
    single-bank Gram, then the combo matmuls above.
  - Output is one 8-byte DMA of [min_k, E0*E1] on the idle SP queue.

Raw Bass: single sync-wait slot per instruction; each engine runs a
hand-scheduled in-order program with explicit cross-engine waits.
GPSIMD cannot access PSUM, so all PSUM-side elementwise work is DVE's.
"""

import dataclasses
import itertools
from contextlib import ExitStack

import ml_dtypes
import numpy as np

from concourse import bass, mybir
from concourse.bass_utils import run_bass_kernel_spmd

F32 = mybir.dt.float32
BF16 = mybir.dt.bfloat16
FP8 = mybir.dt.float8e4

B = 8
M = 8  # sources
NMIX = 2
NSIG = M + NMIX  # 10 signals stacked: sources then mixtures
T = 32000
P = 128
NCHUNK = T // P  # 250 elements per partition per signal
LBLK = 5  # i-values per Gram block (5*10 = 50 live cols; DoubleRow pairs two)
NBLK = NCHUNK // LBLK  # 50 Gram blocks
LIVE = NSIG * LBLK  # 50 live columns per block
BW = 64  # block width padded to 64: the fp8 DoubleRow ISA check requires out
#         columns in {32, 64} and contiguous k-tiles, so each block carries
#         14 zero columns (they only feed Gram rows/cols the mask zeroes)
DCOLS = NBLK * BW  # 3200 data columns incl. padding
NPAIR = NBLK // 2
# four data waves, all on the SP HWDGE queue, sized to the measured sem
# cadence (~9.2/9.75/10.65/11.5us) so PE never starves between waves
PAIR_EDGES = [0, 7, 14, 20, NPAIR]
K = 2**M - 2  # 254 assignment combos
LOG10_SCALE = 10.0 / float(np.log(10.0))

N_WARMUP = 36  # PE warm-up matmuls sized to end just before wave 0's sem

# U columns (bf16): u1 K | pad2 | u0 K | e8 | e9  -> halves split at 256
UCOLS = 512
UH = UCOLS // 2
U0OFF = UH  # u0 starts exactly at the half boundary
# cst columns: mask100 | U | ones100
UOFF = BW
ONESC = UOFF + UCOLS
CST_COLS = ONESC + 1


def _assignment_matrix() -> np.ndarray:
    """[M, K] f32: a1[m, k] = 1 if source m goes to mixture 1 under combo k."""
    cols = [a for a in itertools.product([0, 1], repeat=M) if 0 < sum(a) < M]
    return np.array(cols, dtype=np.float32).T.copy()


def _const_matrix() -> np.ndarray:
    c = np.zeros((BW, CST_COLS), dtype=np.float32)
    # block-diagonal mask: 1 where both row and col fall in the same i-block
    # (rows/cols 50-63 are the zero pad and stay masked out)
    for i in range(LBLK):
        c[i * NSIG : (i + 1) * NSIG, i * NSIG : (i + 1) * NSIG] = 1.0
    a1 = _assignment_matrix()  # [M, K]
    u = np.zeros((NSIG, UCOLS), dtype=np.float32)
    u[:M, 0:K] = a1  # u1 = [a1; 0; -1]
    u[M + 1, 0:K] = -1.0
    u[:M, U0OFF : U0OFF + K] = 1.0 - a1  # u0 = [1-a1; -1; 0]
    u[M, U0OFF : U0OFF + K] = -1.0
    u[M, U0OFF + K] = 1.0  # e8 -> E0
    u[M + 1, U0OFF + K + 1] = 1.0  # e9 -> E1
    c[:LIVE, UOFF:ONESC] = np.tile(u, (LBLK, 1))
    c[:, ONESC] = 1.0
    return c.astype(ml_dtypes.bfloat16)


def _interleave(sample: np.ndarray) -> np.ndarray:
    """[NSIG, T] f32 -> [P, NBLK*BW] fp8, R[p, b*64+i*10+s] = D[s, p*250+b*5+i],
    cols 50-63 of each 64-col block zero-padded."""
    v = sample.reshape(NSIG, P, NBLK, LBLK).transpose(1, 2, 3, 0)
    z = np.zeros((P, NBLK, BW), dtype=np.float32)
    z[:, :, :LIVE] = np.ascontiguousarray(v).reshape(P, NBLK, LIVE)
    return z.reshape(P, NBLK * BW).astype(ml_dtypes.float8_e4m3)


def _build_kernel() -> bass.Bass:
    nc = bass.Bass(trn_type="TRN2")
    data = nc.declare_dram_parameter("data", [P, DCOLS], FP8, isOutput=False)
    cst = nc.declare_dram_parameter("cst", [BW, CST_COLS], BF16, isOutput=False)
    out = nc.declare_dram_parameter("loss", [1, 2], F32, isOutput=True)

    with ExitStack() as ctx:
        sb = lambda name, shape, dt=F32: ctx.enter_context(
            nc.sbuf_tensor(name, shape, dt)
        )
        ps = lambda name, shape: ctx.enter_context(nc.psum_tensor(name, shape, F32))

        rint = sb("rint", [P, DCOLS], FP8)
        csb = sb("csb", [BW, CST_COLS], BF16)
        wsrc = sb("wsrc", [P, BW], FP8)  # zeroed warm-up operand
        mc = sb("mc", [BW, BW], BF16)
        bufq = sb("bufq", [BW, UCOLS], BF16)
        t2 = sb("t2", [1, UH])  # SBUF copy of P2's ne0|E0|E1 half
        pks = sb("pks", [1, K])
        res2 = sb("res2", [1, 2])  # [min_k ne0*ne1 | E0*E1]

        gp = ps("gp", [BW, BW])
        p1 = ps("p1", [BW, UCOLS])
        p2a = ps("p2a", [1, UH])  # ne1 fold half
        p2b = ps("p2b", [1, UH])  # ne0|E0|E1 fold half (separate bank: DVE
        #   reads it while PE still writes the ne1 half)
        wps = ps("wps", [BW, BW])  # warm-up sink, never read

        dsem_w = [
            ctx.enter_context(nc.semaphore(f"dsem_w{w}")) for w in range(4)
        ]
        dsem_c = ctx.enter_context(nc.semaphore("dsem_c"))
        dsem_out = ctx.enter_context(nc.semaphore("dsem_out"))
        pe_sem = ctx.enter_context(nc.semaphore("pe_sem"))
        dve_sem = ctx.enter_context(nc.semaphore("dve_sem"))
        act_sem = ctx.enter_context(nc.semaphore("act_sem"))

        mask = csb[:, 0:BW]
        ucst = csb[:, UOFF:ONESC]
        ones100 = csb[:, ONESC : ONESC + 1]

        # ---- pre-block: input DMAs on both HWDGE queues + warm-up zero ----
        # Descriptor generation (~625-700ns/instr) runs concurrently with
        # block entry.  SP: data cols [0, 814); ACT: the rest, then the
        # constants (needed latest).  The 14-col SBUF tail the last pair
        # over-reads is zeroed on GPSIMD and fenced into PE via gsem.
        for w in range(4):
            c0 = PAIR_EDGES[w] * 2 * BW
            c1 = PAIR_EDGES[w + 1] * 2 * BW
            nc.sync.dma_start(out=rint[:, c0:c1], in_=data[:, c0:c1]).then_inc(
                dsem_w[w], 16
            )
        # constants ride ACT but only after wave 0's bytes are done, so their
        # fat descriptors never steal DMA-engine turns from the early waves
        nc.scalar.wait_ge(dsem_w[0], 16)
        nc.scalar.dma_start(out=csb[:, :], in_=cst[:, :]).then_inc(dsem_c, 16)
        nc.gpsimd.memset(wsrc[:, :], 0.0)

        block = ctx.enter_context(nc.Block())

        @block.sync
        def _(sync):
            sync.wait_ge(dve_sem, 7)
            # No wait on dsem_out: the 8-byte store lands ~7ns after issue
            # while the block-exit barrier takes far longer.
            sync.dma_start(out=out[:, :], in_=res2[:, :]).then_inc(dsem_out, 16)

        @block.vector
        def _(vector):
            vector.wait_ge(dsem_c, 16)
            vector.wait_ge(pe_sem, 25)
            vector.tensor_mul(mc[:, :], gp[:, :], mask).then_inc(dve_sem, 1)    # 1
            vector.wait_ge(pe_sem, 26)
            vector.tensor_mul(
                bufq[:, UH:UCOLS], p1[:, UH:UCOLS], ucst[:, UH:UCOLS]
            ).then_inc(dve_sem, 1)                                              # 2
            vector.wait_ge(pe_sem, 27)
            vector.tensor_mul(
                bufq[:, 0:UH], p1[:, 0:UH], ucst[:, 0:UH]
            ).then_inc(dve_sem, 1)                                              # 3
            vector.wait_ge(pe_sem, 28)
            # walrus: at most one non-scalar PSUM input per DVE op, so stage
            # P2's ne0|E0|E1 half through SBUF; overlaps the other fold half.
            vector.tensor_copy(t2[:, :], p2b[0:1, :]).then_inc(dve_sem, 1)  # 4
            vector.wait_ge(pe_sem, 29)
            vector.wait_ge(dve_sem, 4)
            vector.tensor_mul(
                pks[:, :], p2a[0:1, 0:K], t2[0:1, 0:K]
            ).then_inc(dve_sem, 1)                                              # 5
            vector.tensor_mul(
                res2[0:1, 1:2], t2[0:1, K : K + 1], t2[0:1, K + 1 : K + 2]
            ).then_inc(dve_sem, 1)                                              # 6
            vector.wait_ge(dve_sem, 5)
            vector.tensor_reduce(
                res2[0:1, 0:1], pks[:, :], axis=mybir.AxisListType.X,
                op=mybir.AluOpType.min,
            ).then_inc(dve_sem, 1)                                              # 7

        @block.tensor
        def _(tensor):
            # p-state warm-up on zeros while the data DMA is in flight
            for i in range(N_WARMUP):
                tensor.matmul(
                    wps[:, :], wsrc[:, :], wsrc[:, :],
                    start=(i == 0), stop=(i == N_WARMUP - 1),
                )
            # fp8 DoubleRow: each matmul contracts TWO padded 64-col blocks
            # (~73ns/mm vs ~87ns per single 100-col bf16-rate matmul)
            for w in range(4):
                tensor.wait_ge(dsem_w[w], 16)
                for pr in range(PAIR_EDGES[w], PAIR_EDGES[w + 1]):
                    c0 = pr * 2 * BW
                    pair = rint[:, c0 : c0 + 2 * BW].rearrange(
                        "p (two f) -> p two f", two=2
                    )
                    tensor.matmul(
                        gp[:, :], pair, pair,
                        start=(pr == 0), stop=(pr == NPAIR - 1),
                        perf_mode=mybir.MatmulPerfMode.DoubleRow,
                    ).then_inc(pe_sem, 1)
            # combo stage: P1 = mc^T U, P2 = ones^T (P1 o U), in column
            # halves, ne0|E0|E1 half first so the t2 copy overlaps the rest
            tensor.wait_ge(dve_sem, 1)
            tensor.matmul(
                p1[:, UH:UCOLS], mc[:, :], ucst[:, UH:UCOLS],
                skip_group_check=True,
            ).then_inc(pe_sem, 1)                                               # 26
            tensor.matmul(
                p1[:, 0:UH], mc[:, :], ucst[:, 0:UH], skip_group_check=True
            ).then_inc(pe_sem, 1)                                               # 27
            tensor.wait_ge(dve_sem, 2)
            tensor.matmul(
                p2b[0:1, :], ones100, bufq[:, UH:UCOLS], skip_group_check=True
            ).then_inc(pe_sem, 1)                                               # 28
            tensor.wait_ge(dve_sem, 3)
            tensor.matmul(
                p2a[0:1, :], ones100, bufq[:, 0:UH], skip_group_check=True
            ).then_inc(pe_sem, 1)                                               # 29

    return nc


_NC_CACHE: bass.Bass | None = None


def _in_maps(est: np.ndarray, mx: np.ndarray) -> list[dict]:
    cst = _const_matrix()
    return [
        {
            "data": _interleave(np.concatenate([est[b], mx[b]], axis=0)),
            "cst": cst,
        }
        for b in range(B)
    ]


def kernel(estimated_sources: np.ndarray, input_mixtures: np.ndarray) -> np.ndarray:
    global _NC_CACHE
    assert estimated_sources.shape == (B, M, T)
    assert input_mixtures.shape == (B, NMIX, T)
    if _NC_CACHE is None:
        _NC_CACHE = _build_kernel()
    nc = _NC_CACHE

    est = np.asarray(estimated_sources, dtype=np.float32)
    mx = np.asarray(input_mixtures, dtype=np.float32)
    res = run_bass_kernel_spmd(nc, _in_maps(est, mx), core_ids=list(range(B)))
    # Per-core gather: device ships [min_k ne0*ne1, E0*E1]; fold the logs into
    # the same host reduction that averages the 8 per-sample losses.
    mn = np.array([res.results[b]["loss"][0, 0] for b in range(B)], dtype=np.float64)
    ee = np.array([res.results[b]["loss"][0, 1] for b in range(B)], dtype=np.float64)
    vals = LOG10_SCALE * (np.log(mn) - np.log(ee))
    return np.asarray(vals.mean(), dtype=np.float32)
